# revision 18
# baseline (speedup 1.0000x reference)
"""GRU-D layer kernel for Trainium2, 8 NeuronCores, batch-parallel.

Problem shapes: x [256, 512, 128], h_decay [256, 512], H=256.
Sharding: batch 256 -> 32 per core; GRU weights replicated.

v4 design (pattern-matched to HW microbenchmarks; the recurrence is a
serial cross-engine chain, so the psum handoff shape dominates):
- PER-STEP psum tiles [128, 64] per gate (z, r, h candidate), each fed
  by 7 small matmuls (bias via K=2 selector with start=True, 2 input-
  projection matmuls from the x tile, 4 U-recurrence matmuls), with at
  most 2 column offsets per tile and CONTIGUOUS ACT reads. HW measures
  this pattern at ~40-80 ns per LDW+matmul; 4-step shared psum banks
  with 4 write offsets / strided ACT reads measure ~10x slower.
- r-gate first: sigma_r -> rh -> Uh -> tanh is the critical chain;
  the z-gate matmuls and sigma_z run in the sigma_r/rh latency shadow.
- DVE per step: hdec, rh, a2=(z-1)*hdec, b2=z*hp, h=b2-a2.
- All elementwise tensors bf16 (2x DVE; psum f32). x, dec broadcast,
  output bf16 (halves HBM traffic; validated rel err ~7e-3 vs 2e-2).

Layouts (per core, batch b in 0..31, h chunk c in {0,1},
h index = 128c + p):
  h/hdec/z/r/hp tiles [128, 64] : col = 32c + b
  psum per step per gate [128, 64] f32 : col = 32c + b
  xT  [NT, 128, 128]  bf16 : [nt, d, 32t' + b],  nt = t//4
  decb[NG, 128, 1024] bf16 : [gi, p, 64t'' + 32c + b] (t''=t%16)
  outG[NG, 128, 1024] bf16 : same col layout as decb, value h[b,t,128c+p]
"""

import numpy as np

B, T, D, H = 256, 512, 128, 256
NCORES = 8
BS = B // NCORES  # 32

TRACE = False
LAST_EXEC_NS = None

_NC_CACHE = {}


def _build(T_steps, variant=()):
    vset = set(variant)
    import concourse.bass as bass
    import concourse.mybir as mybir
    from concourse.tile import TileContext

    f32 = mybir.dt.float32
    bf16 = mybir.dt.bfloat16
    SIG = mybir.ActivationFunctionType.Sigmoid
    TANH = mybir.ActivationFunctionType.Tanh
    MUL = mybir.AluOpType.mult
    SUB = mybir.AluOpType.subtract

    NT = T_steps // 4        # x-tiles (4 steps each)
    GS = 16                  # steps per dec/out DMA group
    NG = T_steps // GS

    nc = bass.Bass()
    xT_d = nc.dram_tensor("xT", [128, NT * 128], bf16, kind="ExternalInput")
    w_d = nc.dram_tensor("W6", [128, 768], bf16, kind="ExternalInput")
    u_d = nc.dram_tensor("U12", [128, 1536], bf16, kind="ExternalInput")
    btab_d = nc.dram_tensor("btab", [2, 384], bf16, kind="ExternalInput")
    sel_d = nc.dram_tensor("sel2", [2, 64], bf16, kind="ExternalInput")
    decb_d = nc.dram_tensor("decb", [128, T_steps * 64], bf16,
                            kind="ExternalInput")
    outG_d = nc.dram_tensor("outG", [128, T_steps * 64], bf16,
                            kind="ExternalOutput")

    with TileContext(nc) as tc:
        with (
            tc.tile_pool(name="res", bufs=1) as res,
            tc.tile_pool(name="pz", bufs=2, space="PSUM") as pzp,
            tc.tile_pool(name="pr", bufs=2, space="PSUM") as prp,
            tc.tile_pool(name="ph", bufs=2, space="PSUM") as php,
            tc.tile_pool(name="wk", bufs=16) as wk,
        ):
            # ---- resident constants ----
            w_sb = res.tile([128, 768], bf16)
            nc.sync.dma_start(out=w_sb, in_=w_d[:])
            u_sb = res.tile([128, 1536], bf16)
            nc.sync.dma_start(out=u_sb, in_=u_d[:])
            btab = res.tile([2, 384], bf16)
            nc.sync.dma_start(out=btab, in_=btab_d[:])
            sel2 = res.tile([2, 64], bf16)
            nc.sync.dma_start(out=sel2, in_=sel_d[:])

            def u_blk(gate, mc, kc):
                i0 = ((gate * 2 + mc) * 2 + kc) * 128
                return u_sb[:, i0:i0 + 128]

            def w_blk(gate, c):
                i0 = (2 * gate + c) * 128
                return w_sb[:, i0:i0 + 128]

            def gate_mms(pt, gate, xcol, rhs):
                """bias + proj + U matmuls for one gate into [128, 64]."""
                nc.tensor.matmul(pt[:], btab[:, 128 * gate:128 * gate + 128],
                                 sel2[:], start=True, stop=False)
                for c in range(2):
                    nc.tensor.matmul(pt[:, 32 * c:32 * c + 32],
                                     w_blk(gate, c), xcol,
                                     start=False, stop=False)
                for mc in range(2):
                    for kc in range(2):
                        last = mc == 1 and kc == 1
                        nc.tensor.matmul(
                            pt[:, 32 * mc:32 * mc + 32], u_blk(gate, mc, kc),
                            rhs[:, 32 * kc:32 * kc + 32],
                            start=False, stop=last)

            # ---- prologue: preload x and dec fully into SBUF; the
            # steady-state loop then runs DMA-free (loop DMAs measured
            # ~5 us/step of serialization on HW) ----
            h0 = res.tile([128, 64], bf16)
            nc.any.memzero(h0)
            hdec_state = h0[:]

            xsb = res.tile([128, NT * 128], bf16)
            decsb = res.tile([128, T_steps * 64], bf16)
            outsb = res.tile([128, T_steps * 64], bf16)
            NCOL = NT * 128
            for a in range(0, NCOL, NCOL // 4):
                b_ = min(NCOL, a + NCOL // 4)
                nc.sync.dma_start(out=xsb[:, a:b_], in_=xT_d[:, a:b_])
            DCOL = T_steps * 64
            for a in range(0, DCOL, DCOL // 4):
                b_ = min(DCOL, a + DCOL // 4)
                nc.sync.dma_start(out=decsb[:, a:b_], in_=decb_d[:, a:b_])

            # ---- main loop ----
            OCH = 128 if T_steps >= 128 else T_steps  # steps per out DMA
            for t in range(T_steps):
                j, tp = t // 4, t % 4
                gi, tg = t // GS, t % GS

                # hdec for step t was produced at the end of step t-1
                # (hdec_{t+1} = (z*dec')*hp - dec'*a2, by linearity); step 0
                # starts from the zeroed h0 tile.
                hdec = hdec_state

                # 2) r gate (critical path), then z gate
                xcol = xsb[:, 128 * j + 32 * tp:128 * j + 32 * tp + 32]
                pr = prp.tile([128, 64], f32, tag="pr", name="pr")
                gate_mms(pr, 1, xcol, hdec)
                pz = pzp.tile([128, 64], f32, tag="pz", name="pz")
                gate_mms(pz, 0, xcol, hdec)

                # 3) sigmoids (r first; z runs in the rh latency shadow)
                r_ = wk.tile([128, 64], bf16, tag="rr")
                nc.scalar.activation(out=r_, in_=pr[:], func=SIG)
                z_ = wk.tile([128, 64], bf16, tag="zz")
                nc.scalar.activation(out=z_, in_=pz[:], func=SIG)

                # 4) rh = r * hdec (DVE, on-chain);
                #    a2 = (z-1)*hdec and the dec'-scaled helpers on GPSIMD
                rh = wk.tile([128, 64], bf16, tag="rh")
                nc.vector.tensor_tensor(out=rh, in0=r_[:], in1=hdec[:],
                                        op=MUL)
                a2 = wk.tile([128, 64], bf16, tag="a2")
                nc.vector.scalar_tensor_tensor(out=a2, in0=z_[:], scalar=1.0,
                                               in1=hdec[:], op0=SUB, op1=MUL)
                dbn = decsb[:, 64 * (t + 1):64 * (t + 1) + 64] \
                    if t + 1 < T_steps else decsb[:, 0:64]
                zd = wk.tile([128, 64], bf16, tag="zd")
                nc.gpsimd.tensor_tensor(out=zd, in0=z_[:], in1=dbn, op=MUL)
                da2 = wk.tile([128, 64], bf16, tag="da2")
                nc.gpsimd.tensor_tensor(out=da2, in0=a2[:], in1=dbn, op=MUL)

                # 5) candidate gate
                ph = php.tile([128, 64], f32, tag="ph", name="ph")
                gate_mms(ph, 2, xcol, rh)

                # 6) tanh
                hp = wk.tile([128, 64], bf16, tag="hp")
                nc.scalar.activation(out=hp, in_=ph[:], func=TANH)

                # 7) chain: hdec_{t+1} = zd*hp - da2  (2 DVE ops);
                #    output h = z*hp - a2 off-chain on GPSIMD
                m_ = wk.tile([128, 64], bf16, tag="m")
                nc.vector.tensor_tensor(out=m_, in0=zd[:], in1=hp[:], op=MUL)
                hdn = wk.tile([128, 64], bf16, tag="hdn")
                nc.vector.tensor_tensor(out=hdn, in0=m_[:], in1=da2[:],
                                        op=SUB)
                hdec_state = hdn[:]
                b2 = wk.tile([128, 64], bf16, tag="b2")
                nc.gpsimd.tensor_tensor(out=b2, in0=z_[:], in1=hp[:], op=MUL)
                h_new = outsb[:, 64 * t:64 * t + 64]
                nc.gpsimd.tensor_tensor(out=h_new, in0=b2[:], in1=a2[:],
                                        op=SUB)

                if t % OCH == OCH - 1 and "no_out" not in vset:
                    a = 64 * (t + 1 - OCH)
                    b_ = 64 * (t + 1)
                    nc.sync.dma_start(out=outG_d[:, a:b_],
                                      in_=outsb[:, a:b_])

    _split_matmul_waits(nc, mybir)
    return nc


def _split_matmul_waits(nc, mybir):
    """Walrus allows at most one sync wait per engine instruction. Move the
    excess onto same-engine NoOps inserted just before (avoids
    InstEventSemaphore, which is subject to the cayman event-accel
    deadlock)."""
    for func in nc.m.functions:
        for blk in func.blocks:
            new_insts = []
            for inst in blk.instructions:
                si = inst.sync_info
                if si is not None and len(si.on_wait) > 1:
                    extra = list(si.on_wait[:-1])
                    keep = [si.on_wait[-1]]
                    for w in extra:
                        nop = mybir.InstNoOp(
                            name=nc.get_next_instruction_name(),
                            sync_info=mybir.SyncInfo(on_wait=[w], on_update=[]),
                            engine=inst.engine,
                            bass_nofuse=True,
                        )
                        nc.register_instruction(nop)
                        new_insts.append(nop)
                    si.on_wait = keep
                new_insts.append(inst)
            blk.instructions[:] = new_insts


def _get_nc(T_steps=T, variant=()):
    key = (T_steps, tuple(variant))
    if key not in _NC_CACHE:
        _NC_CACHE[key] = _build(T_steps, variant)
    return _NC_CACHE[key]


def _prep_shared(Wr, Wz, Wh, Ur, Uz, Uh, br, bz, bh):
    import ml_dtypes
    bf = ml_dtypes.bfloat16
    Ws = [np.asarray(a, np.float32) for a in (Wz, Wr, Wh)]
    Us = [np.asarray(a, np.float32) for a in (Uz, Ur, Uh)]
    bs = [np.asarray(a, np.float32) for a in (bz, br, bh)]

    W6 = np.empty((128, 768), np.float32)
    for gate in range(3):
        for c in range(2):
            W6[:, (2 * gate + c) * 128:(2 * gate + c + 1) * 128] = \
                Ws[gate][:, 128 * c:128 * c + 128]

    U12 = np.empty((128, 1536), np.float32)
    for gate in range(3):
        for mc in range(2):
            for kc in range(2):
                i0 = ((gate * 2 + mc) * 2 + kc) * 128
                U12[:, i0:i0 + 128] = Us[gate][128 * kc:128 * kc + 128,
                                               128 * mc:128 * mc + 128]

    # btab[c, 128*gate + m] = b_gate[128c + m]
    btab = np.empty((2, 384), np.float32)
    for gate in range(3):
        for c in range(2):
            btab[c, 128 * gate:128 * gate + 128] = \
                bs[gate][128 * c:128 * c + 128]

    # sel2[c', 32c + b] = 1 iff c' == c   (bias col selector per gate)
    sel2 = np.zeros((2, 64), np.float32)
    sel2[0, 0:32] = 1.0
    sel2[1, 32:64] = 1.0

    return dict(W6=W6.astype(bf), U12=U12.astype(bf),
                btab=btab.astype(bf), sel2=sel2.astype(bf))


def _prep_core(xs, ds, T_steps):
    import ml_dtypes
    bf = ml_dtypes.bfloat16
    xs = np.asarray(xs, np.float32)   # [32, T, 128]
    ds = np.asarray(ds, np.float32)   # [32, T]
    nt = T_steps // 4
    # xT[p, 128j + 32t' + b] = xs[b, 4j + t', p]
    xr = xs.reshape(BS, nt, 4, 128).transpose(3, 1, 2, 0)
    xT = np.ascontiguousarray(xr.reshape(128, nt * 128)).astype(bf)
    # decb[p, 64t + 32c + b] = ds[b, t]
    dT = ds.T                                     # [t, b]
    db = np.concatenate([dT, dT], axis=1).reshape(1, T_steps * 64)
    decb = np.ascontiguousarray(
        np.broadcast_to(db, (128, T_steps * 64))).astype(bf)
    return dict(xT=xT, decb=decb)


def _run_spmd(nc, in_maps, n_timed=0):
    """Replicates bass2jax.run_bass_via_pjrt's multi-core path, optionally
    re-executing the compiled body with device-resident inputs to measure
    per-run wall time (no NTFF profiling hook exists in this environment)."""
    import time
    import jax
    import jax.numpy as jnp
    from jax.sharding import Mesh, PartitionSpec
    from jax.experimental.shard_map import shard_map
    import concourse.mybir as mybir
    from concourse import bass2jax
    from concourse.bass2jax import _bass_exec_p, partition_id_tensor

    bass2jax.install_neuronx_cc_hook()
    if not nc.is_finalized():
        nc.finalize()

    partition_name = (nc.partition_id_tensor.name
                      if nc.partition_id_tensor else None)
    in_names, out_names, out_avals, zero_outs = [], [], [], []
    for alloc in nc.m.functions[0].allocations:
        if not isinstance(alloc, mybir.MemoryLocationSet):
            continue
        name = alloc.memorylocations[0].name
        if alloc.kind == "ExternalInput":
            if name != partition_name:
                in_names.append(name)
        elif alloc.kind == "ExternalOutput":
            aval = jax.core.ShapedArray(
                tuple(alloc.tensor_shape), mybir.dt.np(alloc.dtype))
            out_names.append(name)
            out_avals.append(aval)
            zero_outs.append(np.zeros(aval.shape, aval.dtype))

    n_params = len(in_names)
    all_names = list(in_names) + list(out_names)
    if partition_name is not None:
        all_names.append(partition_name)

    def _body(*args):
        operands = list(args)
        if partition_name is not None:
            operands.append(partition_id_tensor())
        return tuple(_bass_exec_p.bind(
            *operands,
            out_avals=tuple(out_avals),
            in_names=tuple(all_names),
            out_names=tuple(out_names),
            lowering_input_output_aliases=(),
            sim_require_finite=True,
            sim_require_nnan=True,
            nc=nc,
        ))

    devices = jax.devices()[:NCORES]
    mesh = Mesh(np.asarray(devices), ("core",))
    nio = n_params + len(out_names)
    sharded = jax.jit(shard_map(
        _body, mesh=mesh,
        in_specs=(PartitionSpec("core"),) * nio,
        out_specs=(PartitionSpec("core"),) * len(out_names),
        check_rep=False), keep_unused=True)

    concat_in = [np.concatenate([np.asarray(m[name]) for m in in_maps], axis=0)
                 for name in in_names]
    concat_zeros = [np.zeros((NCORES * z.shape[0], *z.shape[1:]), z.dtype)
                    for z in zero_outs]
    args = concat_in + concat_zeros

    out_arrs = sharded(*args)
    jax.block_until_ready(out_arrs)

    times = []
    if n_timed:
        # Axon dispatch costs ~100ms per blocked round-trip, so time N
        # queued (unblocked) executions and difference totals: the device
        # runs them back-to-back.
        sharding = jax.sharding.NamedSharding(mesh, PartitionSpec("core"))
        dev_args = [jax.device_put(a, sharding) for a in args]
        jax.block_until_ready(dev_args)

        def _timed(n):
            t0 = time.perf_counter()
            o = None
            for _ in range(n):
                o = sharded(*dev_args)
            jax.block_until_ready(o)
            return time.perf_counter() - t0

        _timed(1)  # warm
        t1 = min(_timed(1) for _ in range(6))
        tn = min(_timed(1 + n_timed) for _ in range(3))
        times = [(tn - t1) / n_timed]

    results = [
        {name: np.asarray(out_arrs[i]).reshape(NCORES, *out_avals[i].shape)[c]
         for i, name in enumerate(out_names)}
        for c in range(NCORES)
    ]
    return results, times


def _make_in_maps(x, h_decay, Wr, Wz, Wh, Ur, Uz, Uh, br, bz, bh, T_steps=T):
    shared = _prep_shared(Wr, Wz, Wh, Ur, Uz, Uh, br, bz, bh)
    x = np.asarray(x, np.float32)
    h_decay = np.asarray(h_decay, np.float32)
    in_maps = []
    for c in range(NCORES):
        m = dict(shared)
        m.update(_prep_core(x[c * BS:(c + 1) * BS],
                            h_decay[c * BS:(c + 1) * BS], T_steps))
        in_maps.append(m)
    return in_maps


def kernel(x, h_decay, Wr, Wz, Wh, Ur, Uz, Uh, br, bz, bh):
    global LAST_EXEC_NS
    nc = _get_nc(T)
    in_maps = _make_in_maps(x, h_decay, Wr, Wz, Wh, Ur, Uz, Uh, br, bz, bh)
    n_timed = 5 if TRACE else 0
    results, times = _run_spmd(nc, in_maps, n_timed=n_timed)
    if times:
        LAST_EXEC_NS = int(min(times) * 1e9)

    out = np.empty((B, T, H), np.float32)
    for c in range(NCORES):
        out[c * BS:(c + 1) * BS] = _unshard_out(results[c]["outG"], T)
    return out


def _unshard_out(oG, T_steps):
    # oG[p, 64t + 32c + b] -> [b, t, h=128c+p]
    o = np.asarray(oG, np.float32).reshape(128, T_steps, 2, BS)
    return o.transpose(3, 1, 2, 0).reshape(BS, T_steps, H)


# revision 19
# speedup vs baseline: 1.0612x; 1.0612x over previous
"""GRU-D layer kernel for Trainium2, 8 NeuronCores, batch-parallel.

Problem shapes: x [256, 512, 128], h_decay [256, 512], H=256.
Sharding: batch 256 -> 32 per core; GRU weights replicated.

v6 design (pattern-matched to HW microbenchmarks; the recurrence is a
serial cross-engine chain, so per-step chain latency dominates):
- x, dec, and the output all stay SBUF-resident: contiguous bulk DMAs
  in the prologue/epilogue only, a DMA-free steady-state loop.
- hdec_{t+1} = (z*dec')*hp - dec'*a2 by linearity: two DVE ops straight
  off tanh; the output h = z*hp - a2 is computed off-chain on GPSIMD.
- PER-STEP psum tiles [128, 64] per gate (z, r, h candidate), each fed
  by 7 small matmuls (bias via K=2 selector with start=True, 2 input-
  projection matmuls from the x tile, 4 U-recurrence matmuls), with at
  most 2 column offsets per tile and CONTIGUOUS ACT reads. HW measures
  this pattern at ~40-80 ns per LDW+matmul; 4-step shared psum banks
  with 4 write offsets / strided ACT reads measure ~10x slower.
- r-gate first: sigma_r -> rh -> Uh -> tanh is the critical chain;
  the z-gate matmuls and sigma_z run in the sigma_r/rh latency shadow.
- DVE per step: hdec, rh, a2=(z-1)*hdec, b2=z*hp, h=b2-a2.
- All elementwise tensors bf16 (2x DVE; psum f32). x, dec broadcast,
  output bf16 (halves HBM traffic; validated rel err ~7e-3 vs 2e-2).

Layouts (per core, batch b in 0..31, h chunk c in {0,1},
h index = 128c + p):
  h/hdec/z/r/hp tiles [128, 64] : col = 32c + b
  psum per step per gate [128, 64] f32 : col = 32c + b
  xT  [NT, 128, 128]  bf16 : [nt, d, 32t' + b],  nt = t//4
  decb[NG, 128, 1024] bf16 : [gi, p, 64t'' + 32c + b] (t''=t%16)
  outG[NG, 128, 1024] bf16 : same col layout as decb, value h[b,t,128c+p]
"""

import numpy as np

B, T, D, H = 256, 512, 128, 256
NCORES = 8
BS = B // NCORES  # 32

TRACE = False
LAST_EXEC_NS = None

_NC_CACHE = {}


def _build(T_steps, variant=()):
    vset = set(variant)
    import concourse.bass as bass
    import concourse.mybir as mybir
    from concourse.tile import TileContext

    f32 = mybir.dt.float32
    bf16 = mybir.dt.bfloat16
    SIG = mybir.ActivationFunctionType.Sigmoid
    TANH = mybir.ActivationFunctionType.Tanh
    MUL = mybir.AluOpType.mult
    SUB = mybir.AluOpType.subtract

    NT = T_steps // 4        # x-tiles (4 steps each)
    GS = 16                  # steps per dec/out DMA group
    NG = T_steps // GS

    nc = bass.Bass()
    xT_d = nc.dram_tensor("xT", [128, NT * 128], bf16, kind="ExternalInput")
    w_d = nc.dram_tensor("W6", [128, 768], bf16, kind="ExternalInput")
    u_d = nc.dram_tensor("U12", [128, 1536], bf16, kind="ExternalInput")
    btab_d = nc.dram_tensor("btab", [2, 384], bf16, kind="ExternalInput")
    sel_d = nc.dram_tensor("sel2", [2, 64], bf16, kind="ExternalInput")
    decb_d = nc.dram_tensor("decb", [128, T_steps * 64], bf16,
                            kind="ExternalInput")
    outG_d = nc.dram_tensor("outG", [128, T_steps * 64], bf16,
                            kind="ExternalOutput")

    with TileContext(nc) as tc:
        with (
            tc.tile_pool(name="res", bufs=1) as res,
            tc.tile_pool(name="pz", bufs=2, space="PSUM") as pzp,
            tc.tile_pool(name="pr", bufs=2, space="PSUM") as prp,
            tc.tile_pool(name="ph", bufs=2, space="PSUM") as php,
            tc.tile_pool(name="wk", bufs=8) as wk,
        ):
            # ---- resident constants ----
            w_sb = res.tile([128, 768], bf16)
            nc.sync.dma_start(out=w_sb, in_=w_d[:])
            u_sb = res.tile([128, 1536], bf16)
            nc.sync.dma_start(out=u_sb, in_=u_d[:])
            btab = res.tile([2, 384], bf16)
            nc.sync.dma_start(out=btab, in_=btab_d[:])
            sel2 = res.tile([2, 64], bf16)
            nc.sync.dma_start(out=sel2, in_=sel_d[:])

            def u_blk(gate, mc, kc):
                i0 = ((gate * 2 + mc) * 2 + kc) * 128
                return u_sb[:, i0:i0 + 128]

            def w_blk(gate, c):
                i0 = (2 * gate + c) * 128
                return w_sb[:, i0:i0 + 128]

            def gate_mms(pt, gate, xcol, rhs):
                """bias + proj + U matmuls for one gate into [128, 64]."""
                nc.tensor.matmul(pt[:], btab[:, 128 * gate:128 * gate + 128],
                                 sel2[:], start=True, stop=False)
                for c in range(2):
                    nc.tensor.matmul(pt[:, 32 * c:32 * c + 32],
                                     w_blk(gate, c), xcol,
                                     start=False, stop=False)
                for mc in range(2):
                    for kc in range(2):
                        last = mc == 1 and kc == 1
                        nc.tensor.matmul(
                            pt[:, 32 * mc:32 * mc + 32], u_blk(gate, mc, kc),
                            rhs[:, 32 * kc:32 * kc + 32],
                            start=False, stop=last)

            # ---- prologue: preload x and dec fully into SBUF; the
            # steady-state loop then runs DMA-free (loop DMAs measured
            # ~5 us/step of serialization on HW) ----
            h0 = res.tile([128, 64], bf16)
            nc.any.memzero(h0)
            hdec_state = h0[:]

            xsb = res.tile([128, NT * 128], bf16)
            decsb = res.tile([128, T_steps * 64], bf16)
            outsb = res.tile([128, T_steps * 64], bf16)
            NCOL = NT * 128
            for a in range(0, NCOL, NCOL // 4):
                b_ = min(NCOL, a + NCOL // 4)
                nc.sync.dma_start(out=xsb[:, a:b_], in_=xT_d[:, a:b_])
            DCOL = T_steps * 64
            for a in range(0, DCOL, DCOL // 4):
                b_ = min(DCOL, a + DCOL // 4)
                nc.sync.dma_start(out=decsb[:, a:b_], in_=decb_d[:, a:b_])

            # ---- main loop ----
            OCH = 128 if T_steps >= 128 else T_steps  # steps per out DMA
            for t in range(T_steps):
                j, tp = t // 4, t % 4
                gi, tg = t // GS, t % GS

                # hdec for step t was produced at the end of step t-1
                # (hdec_{t+1} = (z*dec')*hp - dec'*a2, by linearity); step 0
                # starts from the zeroed h0 tile.
                hdec = hdec_state

                # 2) r gate (critical path), then z gate
                xcol = xsb[:, 128 * j + 32 * tp:128 * j + 32 * tp + 32]
                pr = prp.tile([128, 64], f32, tag="pr", name="pr")
                gate_mms(pr, 1, xcol, hdec)
                pz = pzp.tile([128, 64], f32, tag="pz", name="pz")
                gate_mms(pz, 0, xcol, hdec)

                # 3) sigmoids (r first; z runs in the rh latency shadow)
                r_ = wk.tile([128, 64], bf16, tag="rr")
                nc.scalar.activation(out=r_, in_=pr[:], func=SIG)
                z_ = wk.tile([128, 64], bf16, tag="zz")
                nc.scalar.activation(out=z_, in_=pz[:], func=SIG)

                # 4) rh = r * hdec (DVE, on-chain);
                #    a2 = (z-1)*hdec and the dec'-scaled helpers on GPSIMD
                rh = wk.tile([128, 64], bf16, tag="rh")
                nc.vector.tensor_tensor(out=rh, in0=r_[:], in1=hdec[:],
                                        op=MUL)
                a2 = wk.tile([128, 64], bf16, tag="a2")
                nc.vector.scalar_tensor_tensor(out=a2, in0=z_[:], scalar=1.0,
                                               in1=hdec[:], op0=SUB, op1=MUL)
                dbn = decsb[:, 64 * (t + 1):64 * (t + 1) + 64] \
                    if t + 1 < T_steps else decsb[:, 0:64]
                zd = wk.tile([128, 64], bf16, tag="zd")
                nc.gpsimd.tensor_tensor(out=zd, in0=z_[:], in1=dbn, op=MUL)
                da2 = wk.tile([128, 64], bf16, tag="da2")
                nc.gpsimd.tensor_tensor(out=da2, in0=a2[:], in1=dbn, op=MUL)

                # 5) candidate gate
                ph = php.tile([128, 64], f32, tag="ph", name="ph")
                gate_mms(ph, 2, xcol, rh)

                # 6) tanh
                hp = wk.tile([128, 64], bf16, tag="hp")
                nc.scalar.activation(out=hp, in_=ph[:], func=TANH)

                # 7) chain: hdec_{t+1} = zd*hp - da2  (2 DVE ops);
                #    output h = z*hp - a2 off-chain on GPSIMD
                m_ = wk.tile([128, 64], bf16, tag="m")
                nc.vector.tensor_tensor(out=m_, in0=zd[:], in1=hp[:], op=MUL)
                hdn = wk.tile([128, 64], bf16, tag="hdn")
                nc.vector.tensor_tensor(out=hdn, in0=m_[:], in1=da2[:],
                                        op=SUB)
                hdec_state = hdn[:]
                b2 = wk.tile([128, 64], bf16, tag="b2")
                nc.gpsimd.tensor_tensor(out=b2, in0=z_[:], in1=hp[:], op=MUL)
                h_new = outsb[:, 64 * t:64 * t + 64]
                nc.gpsimd.tensor_tensor(out=h_new, in0=b2[:], in1=a2[:],
                                        op=SUB)

                if t % OCH == OCH - 1 and "no_out" not in vset:
                    a = 64 * (t + 1 - OCH)
                    b_ = 64 * (t + 1)
                    nc.sync.dma_start(out=outG_d[:, a:b_],
                                      in_=outsb[:, a:b_])

    _split_matmul_waits(nc, mybir)
    return nc


def _split_matmul_waits(nc, mybir):
    """Walrus allows at most one sync wait per engine instruction. Move the
    excess onto same-engine NoOps inserted just before (avoids
    InstEventSemaphore, which is subject to the cayman event-accel
    deadlock)."""
    for func in nc.m.functions:
        for blk in func.blocks:
            new_insts = []
            for inst in blk.instructions:
                si = inst.sync_info
                if si is not None and len(si.on_wait) > 1:
                    extra = list(si.on_wait[:-1])
                    keep = [si.on_wait[-1]]
                    for w in extra:
                        nop = mybir.InstNoOp(
                            name=nc.get_next_instruction_name(),
                            sync_info=mybir.SyncInfo(on_wait=[w], on_update=[]),
                            engine=inst.engine,
                            bass_nofuse=True,
                        )
                        nc.register_instruction(nop)
                        new_insts.append(nop)
                    si.on_wait = keep
                new_insts.append(inst)
            blk.instructions[:] = new_insts


def _get_nc(T_steps=T, variant=()):
    key = (T_steps, tuple(variant))
    if key not in _NC_CACHE:
        _NC_CACHE[key] = _build(T_steps, variant)
    return _NC_CACHE[key]


def _prep_shared(Wr, Wz, Wh, Ur, Uz, Uh, br, bz, bh):
    import ml_dtypes
    bf = ml_dtypes.bfloat16
    Ws = [np.asarray(a, np.float32) for a in (Wz, Wr, Wh)]
    Us = [np.asarray(a, np.float32) for a in (Uz, Ur, Uh)]
    bs = [np.asarray(a, np.float32) for a in (bz, br, bh)]

    W6 = np.empty((128, 768), np.float32)
    for gate in range(3):
        for c in range(2):
            W6[:, (2 * gate + c) * 128:(2 * gate + c + 1) * 128] = \
                Ws[gate][:, 128 * c:128 * c + 128]

    U12 = np.empty((128, 1536), np.float32)
    for gate in range(3):
        for mc in range(2):
            for kc in range(2):
                i0 = ((gate * 2 + mc) * 2 + kc) * 128
                U12[:, i0:i0 + 128] = Us[gate][128 * kc:128 * kc + 128,
                                               128 * mc:128 * mc + 128]

    # btab[c, 128*gate + m] = b_gate[128c + m]
    btab = np.empty((2, 384), np.float32)
    for gate in range(3):
        for c in range(2):
            btab[c, 128 * gate:128 * gate + 128] = \
                bs[gate][128 * c:128 * c + 128]

    # sel2[c', 32c + b] = 1 iff c' == c   (bias col selector per gate)
    sel2 = np.zeros((2, 64), np.float32)
    sel2[0, 0:32] = 1.0
    sel2[1, 32:64] = 1.0

    return dict(W6=W6.astype(bf), U12=U12.astype(bf),
                btab=btab.astype(bf), sel2=sel2.astype(bf))


def _prep_core(xs, ds, T_steps):
    import ml_dtypes
    bf = ml_dtypes.bfloat16
    xs = np.asarray(xs, np.float32)   # [32, T, 128]
    ds = np.asarray(ds, np.float32)   # [32, T]
    nt = T_steps // 4
    # xT[p, 128j + 32t' + b] = xs[b, 4j + t', p]
    xr = xs.reshape(BS, nt, 4, 128).transpose(3, 1, 2, 0)
    xT = np.ascontiguousarray(xr.reshape(128, nt * 128)).astype(bf)
    # decb[p, 64t + 32c + b] = ds[b, t]
    dT = ds.T                                     # [t, b]
    db = np.concatenate([dT, dT], axis=1).reshape(1, T_steps * 64)
    decb = np.ascontiguousarray(
        np.broadcast_to(db, (128, T_steps * 64))).astype(bf)
    return dict(xT=xT, decb=decb)


def _run_spmd(nc, in_maps, n_timed=0):
    """Replicates bass2jax.run_bass_via_pjrt's multi-core path, optionally
    re-executing the compiled body with device-resident inputs to measure
    per-run wall time (no NTFF profiling hook exists in this environment)."""
    import time
    import jax
    import jax.numpy as jnp
    from jax.sharding import Mesh, PartitionSpec
    from jax.experimental.shard_map import shard_map
    import concourse.mybir as mybir
    from concourse import bass2jax
    from concourse.bass2jax import _bass_exec_p, partition_id_tensor

    bass2jax.install_neuronx_cc_hook()
    if not nc.is_finalized():
        nc.finalize()

    partition_name = (nc.partition_id_tensor.name
                      if nc.partition_id_tensor else None)
    in_names, out_names, out_avals, zero_outs = [], [], [], []
    for alloc in nc.m.functions[0].allocations:
        if not isinstance(alloc, mybir.MemoryLocationSet):
            continue
        name = alloc.memorylocations[0].name
        if alloc.kind == "ExternalInput":
            if name != partition_name:
                in_names.append(name)
        elif alloc.kind == "ExternalOutput":
            aval = jax.core.ShapedArray(
                tuple(alloc.tensor_shape), mybir.dt.np(alloc.dtype))
            out_names.append(name)
            out_avals.append(aval)
            zero_outs.append(np.zeros(aval.shape, aval.dtype))

    n_params = len(in_names)
    all_names = list(in_names) + list(out_names)
    if partition_name is not None:
        all_names.append(partition_name)

    def _body(*args):
        operands = list(args)
        if partition_name is not None:
            operands.append(partition_id_tensor())
        return tuple(_bass_exec_p.bind(
            *operands,
            out_avals=tuple(out_avals),
            in_names=tuple(all_names),
            out_names=tuple(out_names),
            lowering_input_output_aliases=(),
            sim_require_finite=True,
            sim_require_nnan=True,
            nc=nc,
        ))

    devices = jax.devices()[:NCORES]
    mesh = Mesh(np.asarray(devices), ("core",))
    nio = n_params + len(out_names)
    sharded = jax.jit(shard_map(
        _body, mesh=mesh,
        in_specs=(PartitionSpec("core"),) * nio,
        out_specs=(PartitionSpec("core"),) * len(out_names),
        check_rep=False), keep_unused=True)

    concat_in = [np.concatenate([np.asarray(m[name]) for m in in_maps], axis=0)
                 for name in in_names]
    concat_zeros = [np.zeros((NCORES * z.shape[0], *z.shape[1:]), z.dtype)
                    for z in zero_outs]
    args = concat_in + concat_zeros

    out_arrs = sharded(*args)
    jax.block_until_ready(out_arrs)

    times = []
    if n_timed:
        # Axon dispatch costs ~100ms per blocked round-trip, so time N
        # queued (unblocked) executions and difference totals: the device
        # runs them back-to-back.
        sharding = jax.sharding.NamedSharding(mesh, PartitionSpec("core"))
        dev_args = [jax.device_put(a, sharding) for a in args]
        jax.block_until_ready(dev_args)

        def _timed(n):
            t0 = time.perf_counter()
            o = None
            for _ in range(n):
                o = sharded(*dev_args)
            jax.block_until_ready(o)
            return time.perf_counter() - t0

        _timed(1)  # warm
        t1 = min(_timed(1) for _ in range(6))
        tn = min(_timed(1 + n_timed) for _ in range(3))
        times = [(tn - t1) / n_timed]

    results = [
        {name: np.asarray(out_arrs[i]).reshape(NCORES, *out_avals[i].shape)[c]
         for i, name in enumerate(out_names)}
        for c in range(NCORES)
    ]
    return results, times


def _make_in_maps(x, h_decay, Wr, Wz, Wh, Ur, Uz, Uh, br, bz, bh, T_steps=T):
    shared = _prep_shared(Wr, Wz, Wh, Ur, Uz, Uh, br, bz, bh)
    x = np.asarray(x, np.float32)
    h_decay = np.asarray(h_decay, np.float32)
    in_maps = []
    for c in range(NCORES):
        m = dict(shared)
        m.update(_prep_core(x[c * BS:(c + 1) * BS],
                            h_decay[c * BS:(c + 1) * BS], T_steps))
        in_maps.append(m)
    return in_maps


def kernel(x, h_decay, Wr, Wz, Wh, Ur, Uz, Uh, br, bz, bh):
    global LAST_EXEC_NS
    nc = _get_nc(T)
    in_maps = _make_in_maps(x, h_decay, Wr, Wz, Wh, Ur, Uz, Uh, br, bz, bh)
    n_timed = 5 if TRACE else 0
    results, times = _run_spmd(nc, in_maps, n_timed=n_timed)
    if times:
        LAST_EXEC_NS = int(min(times) * 1e9)

    out = np.empty((B, T, H), np.float32)
    for c in range(NCORES):
        out[c * BS:(c + 1) * BS] = _unshard_out(results[c]["outG"], T)
    return out


def _unshard_out(oG, T_steps):
    # oG[p, 64t + 32c + b] -> [b, t, h=128c+p]
    o = np.asarray(oG, np.float32).reshape(128, T_steps, 2, BS)
    return o.transpose(3, 1, 2, 0).reshape(BS, T_steps, H)


# revision 20
# speedup vs baseline: 4.1419x; 3.9032x over previous
"""GRU-D layer kernel for Trainium2, 8 NeuronCores, batch-parallel.

Problem shapes: x [256, 512, 128], h_decay [256, 512], H=256.
Sharding: batch 256 -> 32 per core; GRU weights replicated.

v6 design (pattern-matched to HW microbenchmarks; the recurrence is a
serial cross-engine chain, so per-step chain latency dominates):
- x, dec, and the output all stay SBUF-resident: contiguous bulk DMAs
  in the prologue/epilogue only, a DMA-free steady-state loop.
- hdec_{t+1} = (z*dec')*hp - dec'*a2 by linearity: two DVE ops straight
  off tanh; the output h = z*hp - a2 is computed off-chain on GPSIMD.
- PER-STEP psum tiles [128, 64] per gate (z, r, h candidate), each fed
  by 7 small matmuls (bias via K=2 selector with start=True, 2 input-
  projection matmuls from the x tile, 4 U-recurrence matmuls), with at
  most 2 column offsets per tile and CONTIGUOUS ACT reads. HW measures
  this pattern at ~40-80 ns per LDW+matmul; 4-step shared psum banks
  with 4 write offsets / strided ACT reads measure ~10x slower.
- r-gate first: sigma_r -> rh -> Uh -> tanh is the critical chain;
  the z-gate matmuls and sigma_z run in the sigma_r/rh latency shadow.
- DVE per step: hdec, rh, a2=(z-1)*hdec, b2=z*hp, h=b2-a2.
- All elementwise tensors bf16 (2x DVE; psum f32). x, dec broadcast,
  output bf16 (halves HBM traffic; validated rel err ~7e-3 vs 2e-2).

Layouts (per core, batch b in 0..31, h chunk c in {0,1},
h index = 128c + p):
  h/hdec/z/r/hp tiles [128, 64] : col = 32c + b
  psum per step per gate [128, 64] f32 : col = 32c + b
  xT  [NT, 128, 128]  bf16 : [nt, d, 32t' + b],  nt = t//4
  decb[NG, 128, 1024] bf16 : [gi, p, 64t'' + 32c + b] (t''=t%16)
  outG[NG, 128, 1024] bf16 : same col layout as decb, value h[b,t,128c+p]
"""

import numpy as np

B, T, D, H = 256, 512, 128, 256
NCORES = 8
BS = B // NCORES  # 32

TRACE = False
LAST_EXEC_NS = None

_NC_CACHE = {}


def _build(T_steps, variant=()):
    vset = set(variant)
    import concourse.bass as bass
    import concourse.mybir as mybir
    from concourse.tile import TileContext

    f32 = mybir.dt.float32
    bf16 = mybir.dt.bfloat16
    SIG = mybir.ActivationFunctionType.Sigmoid
    TANH = mybir.ActivationFunctionType.Tanh
    MUL = mybir.AluOpType.mult
    SUB = mybir.AluOpType.subtract

    NT = T_steps // 4        # x-tiles (4 steps each)
    GS = 16                  # steps per dec/out DMA group
    NG = T_steps // GS

    nc = bass.Bass()
    xT_d = nc.dram_tensor("xT", [128, NT * 128], bf16, kind="ExternalInput")
    w_d = nc.dram_tensor("W6", [128, 768], bf16, kind="ExternalInput")
    u_d = nc.dram_tensor("U12", [128, 1536], bf16, kind="ExternalInput")
    btab_d = nc.dram_tensor("btab", [2, 384], bf16, kind="ExternalInput")
    sel_d = nc.dram_tensor("sel2", [2, 64], bf16, kind="ExternalInput")
    decb_d = nc.dram_tensor("decb", [128, T_steps * 64], bf16,
                            kind="ExternalInput")
    outG_d = nc.dram_tensor("outG", [128, T_steps * 64], bf16,
                            kind="ExternalOutput")

    with TileContext(nc) as tc:
        with (
            tc.tile_pool(name="res", bufs=1) as res,
            tc.tile_pool(name="pz", bufs=2, space="PSUM") as pzp,
            tc.tile_pool(name="pr", bufs=2, space="PSUM") as prp,
            tc.tile_pool(name="ph", bufs=2, space="PSUM") as php,
            tc.tile_pool(name="wk", bufs=8) as wk,
        ):
            # ---- resident constants ----
            w_sb = res.tile([128, 768], bf16)
            nc.sync.dma_start(out=w_sb, in_=w_d[:])
            u_sb = res.tile([128, 1536], bf16)
            nc.sync.dma_start(out=u_sb, in_=u_d[:])
            btab = res.tile([2, 384], bf16)
            nc.sync.dma_start(out=btab, in_=btab_d[:])
            sel2 = res.tile([2, 64], bf16)
            nc.sync.dma_start(out=sel2, in_=sel_d[:])

            def u_blk(gate, mc, kc):
                i0 = ((gate * 2 + mc) * 2 + kc) * 128
                return u_sb[:, i0:i0 + 128]

            def w_blk(gate, c):
                i0 = (2 * gate + c) * 128
                return w_sb[:, i0:i0 + 128]

            def gate_mms(pt, gate, xcol, rhs):
                """bias + proj + U matmuls for one gate into [128, 64]."""
                nc.tensor.matmul(pt[:], btab[:, 128 * gate:128 * gate + 128],
                                 sel2[:], start=True, stop=False)
                for c in range(2):
                    nc.tensor.matmul(pt[:, 32 * c:32 * c + 32],
                                     w_blk(gate, c), xcol,
                                     start=False, stop=False)
                for mc in range(2):
                    for kc in range(2):
                        last = mc == 1 and kc == 1
                        nc.tensor.matmul(
                            pt[:, 32 * mc:32 * mc + 32], u_blk(gate, mc, kc),
                            rhs[:, 32 * kc:32 * kc + 32],
                            start=False, stop=last)

            # ---- prologue: preload x and dec fully into SBUF; the
            # steady-state loop then runs DMA-free (loop DMAs measured
            # ~5 us/step of serialization on HW) ----
            h0 = res.tile([128, 64], bf16)
            nc.any.memzero(h0)
            hdec_state = h0[:]

            xsb = res.tile([128, NT * 128], bf16)
            decsb = res.tile([128, T_steps * 64], bf16)
            outsb = res.tile([128, T_steps * 64], bf16)
            NCOL = NT * 128
            for a in range(0, NCOL, NCOL // 4):
                b_ = min(NCOL, a + NCOL // 4)
                nc.sync.dma_start(out=xsb[:, a:b_], in_=xT_d[:, a:b_])
            DCOL = T_steps * 64
            for a in range(0, DCOL, DCOL // 4):
                b_ = min(DCOL, a + DCOL // 4)
                nc.sync.dma_start(out=decsb[:, a:b_], in_=decb_d[:, a:b_])

            # ---- main loop ----
            OCH = 128 if T_steps >= 128 else T_steps  # steps per out DMA
            for t in range(T_steps):
                j, tp = t // 4, t % 4
                gi, tg = t // GS, t % GS

                # hdec for step t was produced at the end of step t-1
                # (hdec_{t+1} = (z*dec')*hp - dec'*a2, by linearity); step 0
                # starts from the zeroed h0 tile.
                hdec = hdec_state

                # 2) r gate (critical path), then z gate
                xcol = xsb[:, 128 * j + 32 * tp:128 * j + 32 * tp + 32]
                pr = prp.tile([128, 64], f32, tag="pr", name="pr")
                gate_mms(pr, 1, xcol, hdec)
                pz = pzp.tile([128, 64], f32, tag="pz", name="pz")
                gate_mms(pz, 0, xcol, hdec)

                # 3) sigmoids (r first; z runs in the rh latency shadow)
                r_ = wk.tile([128, 64], bf16, tag="rr")
                nc.scalar.activation(out=r_, in_=pr[:], func=SIG)
                z_ = wk.tile([128, 64], bf16, tag="zz")
                nc.scalar.activation(out=z_, in_=pz[:], func=SIG)

                # 4) rh = r * hdec (DVE, on-chain);
                #    a2 = (z-1)*hdec and the dec'-scaled helpers on GPSIMD
                rh = wk.tile([128, 64], bf16, tag="rh")
                nc.vector.tensor_tensor(out=rh, in0=r_[:], in1=hdec[:],
                                        op=MUL)
                a2 = wk.tile([128, 64], bf16, tag="a2")
                nc.vector.scalar_tensor_tensor(out=a2, in0=z_[:], scalar=1.0,
                                               in1=hdec[:], op0=SUB, op1=MUL)
                dbn = decsb[:, 64 * (t + 1):64 * (t + 1) + 64] \
                    if t + 1 < T_steps else decsb[:, 0:64]
                zd = wk.tile([128, 64], bf16, tag="zd")
                nc.vector.tensor_tensor(out=zd, in0=z_[:], in1=dbn, op=MUL)
                da2 = wk.tile([128, 64], bf16, tag="da2")
                nc.vector.tensor_tensor(out=da2, in0=a2[:], in1=dbn, op=MUL)

                # 5) candidate gate
                ph = php.tile([128, 64], f32, tag="ph", name="ph")
                gate_mms(ph, 2, xcol, rh)

                # 6) tanh
                hp = wk.tile([128, 64], bf16, tag="hp")
                nc.scalar.activation(out=hp, in_=ph[:], func=TANH)

                # 7) chain: hdec_{t+1} = zd*hp - da2  (2 DVE ops);
                #    output h = z*hp - a2 off-chain on GPSIMD
                m_ = wk.tile([128, 64], bf16, tag="m")
                nc.vector.tensor_tensor(out=m_, in0=zd[:], in1=hp[:], op=MUL)
                hdn = wk.tile([128, 64], bf16, tag="hdn")
                nc.vector.tensor_tensor(out=hdn, in0=m_[:], in1=da2[:],
                                        op=SUB)
                hdec_state = hdn[:]
                b2 = wk.tile([128, 64], bf16, tag="b2")
                nc.gpsimd.tensor_tensor(out=b2, in0=z_[:], in1=hp[:], op=MUL)
                h_new = outsb[:, 64 * t:64 * t + 64]
                nc.gpsimd.tensor_tensor(out=h_new, in0=b2[:], in1=a2[:],
                                        op=SUB)

                if t % OCH == OCH - 1 and "no_out" not in vset:
                    a = 64 * (t + 1 - OCH)
                    b_ = 64 * (t + 1)
                    nc.sync.dma_start(out=outG_d[:, a:b_],
                                      in_=outsb[:, a:b_])

    _split_matmul_waits(nc, mybir)
    return nc


def _split_matmul_waits(nc, mybir):
    """Walrus allows at most one sync wait per engine instruction. Move the
    excess onto same-engine NoOps inserted just before (avoids
    InstEventSemaphore, which is subject to the cayman event-accel
    deadlock)."""
    for func in nc.m.functions:
        for blk in func.blocks:
            new_insts = []
            for inst in blk.instructions:
                si = inst.sync_info
                if si is not None and len(si.on_wait) > 1:
                    extra = list(si.on_wait[:-1])
                    keep = [si.on_wait[-1]]
                    for w in extra:
                        nop = mybir.InstNoOp(
                            name=nc.get_next_instruction_name(),
                            sync_info=mybir.SyncInfo(on_wait=[w], on_update=[]),
                            engine=inst.engine,
                            bass_nofuse=True,
                        )
                        nc.register_instruction(nop)
                        new_insts.append(nop)
                    si.on_wait = keep
                new_insts.append(inst)
            blk.instructions[:] = new_insts


def _get_nc(T_steps=T, variant=()):
    key = (T_steps, tuple(variant))
    if key not in _NC_CACHE:
        _NC_CACHE[key] = _build(T_steps, variant)
    return _NC_CACHE[key]


def _prep_shared(Wr, Wz, Wh, Ur, Uz, Uh, br, bz, bh):
    import ml_dtypes
    bf = ml_dtypes.bfloat16
    Ws = [np.asarray(a, np.float32) for a in (Wz, Wr, Wh)]
    Us = [np.asarray(a, np.float32) for a in (Uz, Ur, Uh)]
    bs = [np.asarray(a, np.float32) for a in (bz, br, bh)]

    W6 = np.empty((128, 768), np.float32)
    for gate in range(3):
        for c in range(2):
            W6[:, (2 * gate + c) * 128:(2 * gate + c + 1) * 128] = \
                Ws[gate][:, 128 * c:128 * c + 128]

    U12 = np.empty((128, 1536), np.float32)
    for gate in range(3):
        for mc in range(2):
            for kc in range(2):
                i0 = ((gate * 2 + mc) * 2 + kc) * 128
                U12[:, i0:i0 + 128] = Us[gate][128 * kc:128 * kc + 128,
                                               128 * mc:128 * mc + 128]

    # btab[c, 128*gate + m] = b_gate[128c + m]
    btab = np.empty((2, 384), np.float32)
    for gate in range(3):
        for c in range(2):
            btab[c, 128 * gate:128 * gate + 128] = \
                bs[gate][128 * c:128 * c + 128]

    # sel2[c', 32c + b] = 1 iff c' == c   (bias col selector per gate)
    sel2 = np.zeros((2, 64), np.float32)
    sel2[0, 0:32] = 1.0
    sel2[1, 32:64] = 1.0

    return dict(W6=W6.astype(bf), U12=U12.astype(bf),
                btab=btab.astype(bf), sel2=sel2.astype(bf))


def _prep_core(xs, ds, T_steps):
    import ml_dtypes
    bf = ml_dtypes.bfloat16
    xs = np.asarray(xs, np.float32)   # [32, T, 128]
    ds = np.asarray(ds, np.float32)   # [32, T]
    nt = T_steps // 4
    # xT[p, 128j + 32t' + b] = xs[b, 4j + t', p]
    xr = xs.reshape(BS, nt, 4, 128).transpose(3, 1, 2, 0)
    xT = np.ascontiguousarray(xr.reshape(128, nt * 128)).astype(bf)
    # decb[p, 64t + 32c + b] = ds[b, t]
    dT = ds.T                                     # [t, b]
    db = np.concatenate([dT, dT], axis=1).reshape(1, T_steps * 64)
    decb = np.ascontiguousarray(
        np.broadcast_to(db, (128, T_steps * 64))).astype(bf)
    return dict(xT=xT, decb=decb)


def _run_spmd(nc, in_maps, n_timed=0):
    """Replicates bass2jax.run_bass_via_pjrt's multi-core path, optionally
    re-executing the compiled body with device-resident inputs to measure
    per-run wall time (no NTFF profiling hook exists in this environment)."""
    import time
    import jax
    import jax.numpy as jnp
    from jax.sharding import Mesh, PartitionSpec
    from jax.experimental.shard_map import shard_map
    import concourse.mybir as mybir
    from concourse import bass2jax
    from concourse.bass2jax import _bass_exec_p, partition_id_tensor

    bass2jax.install_neuronx_cc_hook()
    if not nc.is_finalized():
        nc.finalize()

    partition_name = (nc.partition_id_tensor.name
                      if nc.partition_id_tensor else None)
    in_names, out_names, out_avals, zero_outs = [], [], [], []
    for alloc in nc.m.functions[0].allocations:
        if not isinstance(alloc, mybir.MemoryLocationSet):
            continue
        name = alloc.memorylocations[0].name
        if alloc.kind == "ExternalInput":
            if name != partition_name:
                in_names.append(name)
        elif alloc.kind == "ExternalOutput":
            aval = jax.core.ShapedArray(
                tuple(alloc.tensor_shape), mybir.dt.np(alloc.dtype))
            out_names.append(name)
            out_avals.append(aval)
            zero_outs.append(np.zeros(aval.shape, aval.dtype))

    n_params = len(in_names)
    all_names = list(in_names) + list(out_names)
    if partition_name is not None:
        all_names.append(partition_name)

    def _body(*args):
        operands = list(args)
        if partition_name is not None:
            operands.append(partition_id_tensor())
        return tuple(_bass_exec_p.bind(
            *operands,
            out_avals=tuple(out_avals),
            in_names=tuple(all_names),
            out_names=tuple(out_names),
            lowering_input_output_aliases=(),
            sim_require_finite=True,
            sim_require_nnan=True,
            nc=nc,
        ))

    devices = jax.devices()[:NCORES]
    mesh = Mesh(np.asarray(devices), ("core",))
    nio = n_params + len(out_names)
    sharded = jax.jit(shard_map(
        _body, mesh=mesh,
        in_specs=(PartitionSpec("core"),) * nio,
        out_specs=(PartitionSpec("core"),) * len(out_names),
        check_rep=False), keep_unused=True)

    concat_in = [np.concatenate([np.asarray(m[name]) for m in in_maps], axis=0)
                 for name in in_names]
    concat_zeros = [np.zeros((NCORES * z.shape[0], *z.shape[1:]), z.dtype)
                    for z in zero_outs]
    args = concat_in + concat_zeros

    out_arrs = sharded(*args)
    jax.block_until_ready(out_arrs)

    times = []
    if n_timed:
        # Axon dispatch costs ~100ms per blocked round-trip, so time N
        # queued (unblocked) executions and difference totals: the device
        # runs them back-to-back.
        sharding = jax.sharding.NamedSharding(mesh, PartitionSpec("core"))
        dev_args = [jax.device_put(a, sharding) for a in args]
        jax.block_until_ready(dev_args)

        def _timed(n):
            t0 = time.perf_counter()
            o = None
            for _ in range(n):
                o = sharded(*dev_args)
            jax.block_until_ready(o)
            return time.perf_counter() - t0

        _timed(1)  # warm
        t1 = min(_timed(1) for _ in range(6))
        tn = min(_timed(1 + n_timed) for _ in range(3))
        times = [(tn - t1) / n_timed]

    results = [
        {name: np.asarray(out_arrs[i]).reshape(NCORES, *out_avals[i].shape)[c]
         for i, name in enumerate(out_names)}
        for c in range(NCORES)
    ]
    return results, times


def _make_in_maps(x, h_decay, Wr, Wz, Wh, Ur, Uz, Uh, br, bz, bh, T_steps=T):
    shared = _prep_shared(Wr, Wz, Wh, Ur, Uz, Uh, br, bz, bh)
    x = np.asarray(x, np.float32)
    h_decay = np.asarray(h_decay, np.float32)
    in_maps = []
    for c in range(NCORES):
        m = dict(shared)
        m.update(_prep_core(x[c * BS:(c + 1) * BS],
                            h_decay[c * BS:(c + 1) * BS], T_steps))
        in_maps.append(m)
    return in_maps


def kernel(x, h_decay, Wr, Wz, Wh, Ur, Uz, Uh, br, bz, bh):
    global LAST_EXEC_NS
    nc = _get_nc(T)
    in_maps = _make_in_maps(x, h_decay, Wr, Wz, Wh, Ur, Uz, Uh, br, bz, bh)
    n_timed = 5 if TRACE else 0
    results, times = _run_spmd(nc, in_maps, n_timed=n_timed)
    if times:
        LAST_EXEC_NS = int(min(times) * 1e9)

    out = np.empty((B, T, H), np.float32)
    for c in range(NCORES):
        out[c * BS:(c + 1) * BS] = _unshard_out(results[c]["outG"], T)
    return out


def _unshard_out(oG, T_steps):
    # oG[p, 64t + 32c + b] -> [b, t, h=128c+p]
    o = np.asarray(oG, np.float32).reshape(128, T_steps, 2, BS)
    return o.transpose(3, 1, 2, 0).reshape(BS, T_steps, H)
